# revision 4
# baseline (speedup 1.0000x reference)
"""Trainium2 Bass kernel: 2-layer GCN (PyG GCNConv semantics) + global mean
pool + FC, SPMD across 8 NeuronCores.

Plan (single shared instruction stream, per-core data):
- Nodes sharded contiguously: 12500/core, padded to 12544 = 98*128 rows.
- Layer 1 transforms first (h1 = x @ W1 on the shard; host passes x^T so no
  on-chip transposes), AllGather -> full bf16 gather table in DRAM.
- Edges partitioned by destination, sorted; the source table is split into 4
  chunks of 25088 rows so row ids fit dma_gather's int16 indices.  Message
  stream is ordered (chunk-major, dst-tile minor); per (chunk, dst-tile)
  group sizes are padded to multiples of 128 and equalized across cores so
  one program serves all 8.  Rows are fetched with dma_gather (256B rows,
  4096 indices per call, multi-packet).  Per 128-message tile a selection
  matrix S_T[m,d] = (dstloc[m]==d)*norm[m] is built on the vector engine
  from an iota tile; aggregation is a PE matmul accumulating in PSUM, with
  per-chunk partials accumulated into an SBUF accumulator.
  Reference-added self-loops are handled separately: contiguous row loads
  from the core's own shard + diagonal selection (deg^-1), no random DMA.
- Layer 2 aggregates first at 128 features (same edge structure, gathering
  z1), keeping the aggregate transposed, then transforms with W2.
- Mean pool via matmul with Sel_T[n,g] = (batch[n]==g)/cnt[g] accumulated in
  SBUF, AllReduce (64x256 f32), replicated FC + relu.
"""

import numpy as np

import concourse.mybir as mybir
import concourse.tile as tile
from concourse import bacc
from concourse.bass_utils import run_bass_kernel_spmd
from concourse.masks import make_identity

# problem constants (hardcoded per harness contract)
N = 100000
G = 64
IN = 256
H1 = 128
H2 = 256
OUT = 512
NCORES = 8
SH = N // NCORES          # 12500 real nodes per core
NT = (SH + 127) // 128    # 98 dst tiles per core
SHP = NT * 128            # 12544 padded table rows per core
NCH = 4                   # source-table chunks (int16 index reach)
CALL_TILES = 32           # msg tiles per dma_gather call (4096 indices)

BF = np.dtype(mybir.dt.np(mybir.dt.bfloat16))
F32 = mybir.dt.float32
BF16 = mybir.dt.bfloat16
I16 = mybir.dt.int16


def _build(T, Jtot):
    """Build the SPMD program. T: tuple of NCH tuples, T[ch][t] = msg-tile
    count for (chunk ch, dst tile t), identical across cores."""
    CHROWS = SHP * NCORES // NCH  # 25088
    nc = bacc.Bacc("TRN2", target_bir_lowering=False)

    xT = nc.dram_tensor("xT", [IN, SHP], BF16, kind="ExternalInput")
    w1a_d = nc.dram_tensor("w1a", [128, H1], BF16, kind="ExternalInput")
    w1b_d = nc.dram_tensor("w1b", [128, H1], BF16, kind="ExternalInput")
    w2_d = nc.dram_tensor("w2", [H1, H2], BF16, kind="ExternalInput")
    fcwa_d = nc.dram_tensor("fcwa", [128, OUT], BF16, kind="ExternalInput")
    fcwb_d = nc.dram_tensor("fcwb", [128, OUT], BF16, kind="ExternalInput")
    b1_d = nc.dram_tensor("b1", [1, H1], BF16, kind="ExternalInput")
    b2_d = nc.dram_tensor("b2", [1, H2], BF16, kind="ExternalInput")
    fcb_d = nc.dram_tensor("fcb", [1, OUT], BF16, kind="ExternalInput")
    idx_d = nc.dram_tensor("idx16", [128, Jtot * 8], I16, kind="ExternalInput")
    dstl_d = nc.dram_tensor("dstl", [128, Jtot], F32, kind="ExternalInput")
    nrm_d = nc.dram_tensor("nrm", [128, Jtot], F32, kind="ExternalInput")
    dinv2_d = nc.dram_tensor("dinv2", [128, NT], F32, kind="ExternalInput")
    batg_d = nc.dram_tensor("batg", [128, NT], F32, kind="ExternalInput")
    cnti_d = nc.dram_tensor("cnti", [128, NT], F32, kind="ExternalInput")
    out_d = nc.dram_tensor("out", [G, OUT], F32, kind="ExternalOutput")

    RG = [list(range(NCORES))]

    with tile.TileContext(nc) as tc:
        with (
            tc.tile_pool(name="res", bufs=1) as res,
            tc.tile_pool(name="sb", bufs=1) as sb,
            tc.tile_pool(name="ps", bufs=1, space="PSUM") as ps,
            tc.tile_pool(name="dr", bufs=1, space="DRAM") as dr,
        ):
            # resident data
            idx_sb = res.tile([128, Jtot * 8], I16)
            dstl_sb = res.tile([128, Jtot], F32)
            nrm_sb = res.tile([128, Jtot], F32)
            dinv2_sb = res.tile([128, NT], F32)
            batg_sb = res.tile([128, NT], F32)
            cnti_sb = res.tile([128, NT], F32)
            w1a = res.tile([128, H1], BF16)
            w1b = res.tile([128, H1], BF16)
            w2 = res.tile([H1, H2], BF16)
            fcwa = res.tile([128, OUT], BF16)
            fcwb = res.tile([128, OUT], BF16)
            b1s = res.tile([1, H1], BF16)
            b2s = res.tile([1, H2], BF16)
            fcbs = res.tile([1, OUT], BF16)
            for sbuf, dram in (
                (idx_sb, idx_d), (dstl_sb, dstl_d), (nrm_sb, nrm_d),
                (dinv2_sb, dinv2_d), (batg_sb, batg_d), (cnti_sb, cnti_d),
                (w1a, w1a_d), (w1b, w1b_d), (w2, w2_d),
                (fcwa, fcwa_d), (fcwb, fcwb_d),
                (b1s, b1_d), (b2s, b2_d), (fcbs, fcb_d),
            ):
                nc.sync.dma_start(sbuf[:], dram[:])

            # constants
            iota_i = res.tile([128, 128], mybir.dt.int32)
            iota_bf = res.tile([128, 128], BF16)
            cio_i = res.tile([128, 1], mybir.dt.int32)
            cio_f = res.tile([128, 1], F32)
            ones = res.tile([1, 128], BF16)
            ident = res.tile([128, 128], BF16)
            nc.gpsimd.iota(iota_i[:], pattern=[[1, 128]], base=0, channel_multiplier=0)
            nc.vector.tensor_copy(iota_bf[:], iota_i[:])
            nc.gpsimd.iota(cio_i[:], pattern=[[0, 1]], base=0, channel_multiplier=1)
            nc.vector.tensor_copy(cio_f[:], cio_i[:])
            nc.vector.memset(ones[:], 1.0)
            make_identity(nc, ident[:])
            pooled_acc = res.tile([G, H2], F32)
            # per-dst-tile f32 accumulator across chunk passes (reused by both layers)
            acc = res.tile([128, NT * 128], F32)

            # internal DRAM
            h1_shard = dr.tile([SHP, H1], BF16)
            h1_full = dr.tile([SHP * NCORES, H1], BF16, addr_space="Shared")
            z1_shard = dr.tile([SHP, H1], BF16)
            z1_full = dr.tile([SHP * NCORES, H1], BF16, addr_space="Shared")
            pool_part = dr.tile([G, H2], F32)
            pool_red = dr.tile([G, H2], F32, addr_space="Shared")

            # phase A: h1 = x @ W1 on the shard
            for t in range(NT):
                xta = sb.tile([128, 128], BF16, tag="xta", bufs=3)
                xtb = sb.tile([128, 128], BF16, tag="xtb", bufs=3)
                nc.sync.dma_start(xta[:], xT[0:128, t * 128:(t + 1) * 128])
                nc.sync.dma_start(xtb[:], xT[128:256, t * 128:(t + 1) * 128])
                h1p = ps.tile([128, H1], F32, tag="agg", bufs=4, space="PSUM")
                nc.tensor.matmul(h1p[:], lhsT=xta[:], rhs=w1a[:], start=True, stop=False)
                nc.tensor.matmul(h1p[:], lhsT=xtb[:], rhs=w1b[:], start=False, stop=True)
                h1t = sb.tile([128, H1], BF16, tag="h1t", bufs=3)
                nc.scalar.copy(h1t[:], h1p[:])
                nc.sync.dma_start(h1_shard[t * 128:(t + 1) * 128, :], h1t[:])

            nc.gpsimd.collective_compute(
                "AllGather", mybir.AluOpType.bypass, replica_groups=RG,
                ins=[h1_shard.opt()], outs=[h1_full.opt()],
            )

            def msg_pass(layer, table, shard):
                """One GCN aggregation sweep. layer 1: z1 = relu(agg + b1) to
                z1_shard. layer 2: transposed agg -> @W2 + b2, relu, pool."""
                started = [False] * NT
                j = 0  # global msg-tile index
                for ch in range(NCH):
                    tbl = table[ch * CHROWS:(ch + 1) * CHROWS, :]
                    # gather calls for this chunk
                    ch_tiles = sum(T[ch])
                    calls = []
                    o = j
                    while o < j + ch_tiles:
                        nb = min(CALL_TILES, j + ch_tiles - o)
                        calls.append((o, nb))
                        o += nb
                    msgs_cur = (None, 0)  # (tile, base j)
                    for t in range(NT):
                        nt_ch = T[ch][t]
                        if nt_ch == 0 and ch < NCH - 1:
                            continue
                        agg = ps.tile([128, 128], F32, tag="agg", bufs=4, space="PSUM")
                        first_mm = True
                        for i in range(nt_ch):
                            if calls and j == calls[0][0]:
                                o_, nb_ = calls.pop(0)
                                m_t = sb.tile([128, CALL_TILES, 128], BF16,
                                              tag="msgs", bufs=3)
                                nc.gpsimd.dma_gather(
                                    m_t[:, :nb_, :], tbl,
                                    idx_sb[:, o_ * 8:(o_ + nb_) * 8],
                                    nb_ * 128, nb_ * 128, 128,
                                    single_packet=False)
                                msgs_cur = (m_t, o_)
                            st = sb.tile([128, 128], BF16, tag="st", bufs=4)
                            nc.vector.tensor_scalar(
                                out=st[:], in0=iota_bf[:],
                                scalar1=dstl_sb[:, j:j + 1],
                                scalar2=nrm_sb[:, j:j + 1],
                                op0=mybir.AluOpType.is_equal,
                                op1=mybir.AluOpType.mult,
                            )
                            m = msgs_cur[0][:, j - msgs_cur[1], :]
                            last = (ch < NCH - 1) and (i == nt_ch - 1)
                            if layer == 1:
                                nc.tensor.matmul(agg[:], lhsT=st[:], rhs=m,
                                                 start=first_mm, stop=last)
                            else:
                                nc.tensor.matmul(agg[:], lhsT=m, rhs=st[:],
                                                 start=first_mm, stop=last)
                            first_mm = False
                            j += 1
                        if ch < NCH - 1:
                            # bank partial into acc
                            a_sl = acc[:, t * 128:(t + 1) * 128]
                            if not started[t]:
                                nc.vector.tensor_copy(a_sl, agg[:])
                                started[t] = True
                            else:
                                nc.vector.tensor_tensor(
                                    out=a_sl, in0=a_sl, in1=agg[:],
                                    op=mybir.AluOpType.add)
                            continue
                        # final chunk: self-loop + bias, then fold acc and finish
                        srows = sb.tile([128, 128], BF16, tag="srows", bufs=3)
                        nc.sync.dma_start(srows[:], shard[t * 128:(t + 1) * 128, :])
                        sdiag = sb.tile([128, 128], BF16, tag="sdiag", bufs=3)
                        nc.vector.tensor_scalar(
                            out=sdiag[:], in0=iota_bf[:],
                            scalar1=cio_f[:, 0:1], scalar2=dinv2_sb[:, t:t + 1],
                            op0=mybir.AluOpType.is_equal, op1=mybir.AluOpType.mult,
                        )
                        if layer == 1:
                            nc.tensor.matmul(agg[:], lhsT=sdiag[:], rhs=srows[:],
                                             start=first_mm, stop=False)
                            nc.tensor.matmul(agg[:], lhsT=ones[:1, :H1], rhs=b1s[:],
                                             start=False, stop=True)
                        else:
                            nc.tensor.matmul(agg[:], lhsT=srows[:], rhs=sdiag[:],
                                             start=first_mm, stop=True)
                        if started[t]:
                            a_sl = acc[:, t * 128:(t + 1) * 128]
                            nc.vector.tensor_tensor(out=agg[:], in0=agg[:],
                                                    in1=a_sl,
                                                    op=mybir.AluOpType.add)
                        if layer == 1:
                            z1t = sb.tile([128, H1], BF16, tag="z1t", bufs=3)
                            nc.scalar.activation(z1t[:], agg[:],
                                                 mybir.ActivationFunctionType.Relu)
                            nc.sync.dma_start(
                                z1_shard[t * 128:(t + 1) * 128, :], z1t[:])
                        else:
                            g2t = sb.tile([128, 128], BF16, tag="g2t", bufs=3)
                            nc.scalar.copy(g2t[:], agg[:])
                            z2p = ps.tile([128, H2], F32, tag="z2p", bufs=2,
                                          space="PSUM")
                            nc.tensor.matmul(z2p[:], lhsT=g2t[:], rhs=w2[:],
                                             start=True, stop=False)
                            nc.tensor.matmul(z2p[:], lhsT=ones[:1, :128],
                                             rhs=b2s[:], start=False, stop=True)
                            z2t = sb.tile([128, H2], BF16, tag="z2t", bufs=3)
                            nc.scalar.activation(z2t[:], z2p[:],
                                                 mybir.ActivationFunctionType.Relu)
                            selt = sb.tile([128, G], BF16, tag="selt", bufs=3)
                            nc.vector.tensor_scalar(
                                out=selt[:], in0=iota_bf[:, :G],
                                scalar1=batg_sb[:, t:t + 1],
                                scalar2=cnti_sb[:, t:t + 1],
                                op0=mybir.AluOpType.is_equal,
                                op1=mybir.AluOpType.mult,
                            )
                            poolp = ps.tile([G, H2], F32, tag="poolp", bufs=2,
                                            space="PSUM")
                            nc.tensor.matmul(poolp[:], lhsT=selt[:], rhs=z2t[:],
                                             start=True, stop=True)
                            if t == 0:
                                nc.vector.tensor_copy(pooled_acc[:], poolp[:])
                            else:
                                nc.vector.tensor_tensor(
                                    out=pooled_acc[:], in0=pooled_acc[:],
                                    in1=poolp[:], op=mybir.AluOpType.add)

            msg_pass(1, h1_full, h1_shard)
            nc.gpsimd.collective_compute(
                "AllGather", mybir.AluOpType.bypass, replica_groups=RG,
                ins=[z1_shard.opt()], outs=[z1_full.opt()],
            )
            msg_pass(2, z1_full, z1_shard)

            nc.sync.dma_start(pool_part[:], pooled_acc[:])
            nc.gpsimd.collective_compute(
                "AllReduce", mybir.AluOpType.add, replica_groups=RG,
                ins=[pool_part.opt()], outs=[pool_red.opt()],
            )

            # FC (replicated on every core)
            pooled_f = sb.tile([G, H2], F32)
            nc.sync.dma_start(pooled_f[:], pool_red[:])
            pooled_b = sb.tile([G, H2], BF16)
            nc.vector.tensor_copy(pooled_b[:], pooled_f[:])
            pTa = sb.tile([128, G], BF16)
            pTb = sb.tile([128, G], BF16)
            for chunk, pT in ((0, pTa), (1, pTb)):
                tp = ps.tile([128, G], BF16, tag="poolp", bufs=2, space="PSUM")
                nc.tensor.transpose(
                    tp[:], in_=pooled_b[:, chunk * 128:(chunk + 1) * 128],
                    identity=ident[:G, :G])
                nc.scalar.copy(pT[:], tp[:])
            fcp = ps.tile([G, OUT], F32, tag="z2p", bufs=2, space="PSUM")
            nc.tensor.matmul(fcp[:], lhsT=pTa[:], rhs=fcwa[:], start=True, stop=False)
            nc.tensor.matmul(fcp[:], lhsT=pTb[:], rhs=fcwb[:], start=False, stop=False)
            nc.tensor.matmul(fcp[:], lhsT=ones[:1, :G], rhs=fcbs[:],
                             start=False, stop=True)
            out_sb = sb.tile([G, OUT], F32)
            nc.scalar.activation(out_sb[:], fcp[:],
                                 mybir.ActivationFunctionType.Relu)
            nc.sync.dma_start(out_d[:], out_sb[:])

    nc.compile()
    return nc


def _preprocess(x, edge_index, batch, W1, b1, W2, b2, fc_W, fc_b):
    """Host-side index preprocessing; returns (T, Jtot, in_maps)."""
    CHROWS = SHP * NCORES // NCH
    src = np.asarray(edge_index[0], dtype=np.int64)
    dst = np.asarray(edge_index[1], dtype=np.int64)
    batch = np.asarray(batch, dtype=np.int64)

    deg = np.bincount(dst, minlength=N).astype(np.float64) + 1.0  # + self loop
    dinv = 1.0 / np.sqrt(deg)
    norm = (dinv[src] * dinv[dst]).astype(np.float32)

    srow = ((src // SH) * SHP + (src % SH)).astype(np.int64)  # padded table row
    ch_of = srow // CHROWS
    core_of = dst // SH
    tloc = (dst % SH) // 128
    # sort edges by (core, chunk, dst): groups laid out chunk-major per core
    key = ((core_of * NCH + ch_of) * NT + tloc) * N + dst
    order = np.argsort(key, kind="stable")
    src_s, dst_s, nrm_s = srow[order], dst[order], norm[order]
    grp = (core_of * NCH + ch_of)[order] * NT + tloc[order]

    counts = np.bincount(grp, minlength=NCORES * NCH * NT).reshape(NCORES, NCH, NT)
    T = ((counts + 127) // 128).max(axis=0)  # [NCH, NT]
    Jtot = int(T.sum())
    # stream offsets: chunk-major, tile minor
    toff = np.zeros(NCH * NT + 1, np.int64)
    np.cumsum((T * 128).ravel(), out=toff[1:])
    gstart = np.zeros(NCORES * NCH * NT + 1, np.int64)
    np.cumsum(counts.ravel(), out=gstart[1:])

    L = Jtot * 128
    lidx = np.zeros((NCORES, L), np.int64)
    dstl = np.full((NCORES, L), 200.0, np.float32)
    nrmv = np.zeros((NCORES, L), np.float32)
    for c in range(NCORES):
        for chn in range(NCH):
            for t in range(NT):
                g = (c * NCH + chn) * NT + t
                a, b = gstart[g], gstart[g + 1]
                o = toff[chn * NT + t]
                n = b - a
                lidx[c, o:o + n] = src_s[a:b] - chn * CHROWS
                dstl[c, o:o + n] = ((dst_s[a:b] % SH) % 128).astype(np.float32)
                nrmv[c, o:o + n] = nrm_s[a:b]
    # msg s -> [s%128, s//128] for dstl/nrm; idx wraps 16-wide per tile
    dstl = np.ascontiguousarray(dstl.reshape(NCORES, Jtot, 128).transpose(0, 2, 1))
    nrmv = np.ascontiguousarray(nrmv.reshape(NCORES, Jtot, 128).transpose(0, 2, 1))
    idx16 = lidx.reshape(NCORES, Jtot * 8, 16).transpose(0, 2, 1).astype(np.int16)
    idx16 = np.tile(idx16, (1, 8, 1))  # replicate to 128 partitions

    node = np.arange(NCORES * SHP)
    real = (node % SHP) < SH
    gnode = np.where(real, (node // SHP) * SH + (node % SHP), 0)
    dinv2 = np.where(real, dinv[gnode] ** 2, 0.0).astype(np.float32)
    batgl = np.where(real, batch[gnode].astype(np.float64), 200.0).astype(np.float32)
    cnt = np.bincount(batch, minlength=G).astype(np.float64)
    cnti = np.where(real, 1.0 / np.maximum(cnt, 1.0)[batch[gnode]], 0.0).astype(np.float32)

    def to_tiles(a):
        return np.ascontiguousarray(a.reshape(NCORES, NT, 128).transpose(0, 2, 1))

    dinv2, batgl, cnti = to_tiles(dinv2), to_tiles(batgl), to_tiles(cnti)

    x = np.asarray(x, dtype=np.float32)
    xp = np.zeros((NCORES, SHP, IN), np.float32)
    xp[:, :SH, :] = x.reshape(NCORES, SH, IN)
    xT = np.ascontiguousarray(xp.transpose(0, 2, 1)).astype(BF)

    W1 = np.asarray(W1, np.float32).astype(BF)
    W2 = np.asarray(W2, np.float32).astype(BF)
    fc_W = np.asarray(fc_W, np.float32).astype(BF)
    b1 = np.asarray(b1, np.float32).astype(BF).reshape(1, H1)
    b2 = np.asarray(b2, np.float32).astype(BF).reshape(1, H2)
    fc_b = np.asarray(fc_b, np.float32).astype(BF).reshape(1, OUT)

    in_maps = []
    for c in range(NCORES):
        in_maps.append({
            "xT": xT[c],
            "w1a": W1[:128], "w1b": W1[128:], "w2": W2,
            "fcwa": fc_W[:128], "fcwb": fc_W[128:],
            "b1": b1, "b2": b2, "fcb": fc_b,
            "idx16": idx16[c], "dstl": dstl[c], "nrm": nrmv[c],
            "dinv2": dinv2[c], "batg": batgl[c], "cnti": cnti[c],
        })
    return tuple(map(tuple, T.tolist())), Jtot, in_maps


_CACHE = {}


def kernel(**inputs) -> np.ndarray:
    T, Jtot, in_maps = _preprocess(**inputs)
    if T not in _CACHE:
        _CACHE[T] = _build(T, Jtot)
    nc = _CACHE[T]
    r = run_bass_kernel_spmd(nc, in_maps, core_ids=list(range(NCORES)))
    return np.asarray(r.results[0]["out"], dtype=np.float32)
